# revision 1
# baseline (speedup 1.0000x reference)
"""Distributed euclidean-distance loss kernel for Trainium2 (8 NeuronCores).

loss = sum_i sqrt(sum_c (preds[i,c] - targets[i,c])^2) / (N + 1)

preds/targets: [16777216, 2] f32. Data-parallel over the batch axis:
each of the 8 cores reduces its 1/8 slice to a per-partition partial
sum [128, 1]; the host sums the 8x128 partials and divides by N+1.

Host-side sharding packs preds and targets tiles into one interleaved
DRAM tensor per core ([... ntiles, 2, f]) so each SBUF tile is filled
by a single large DMA.
"""

from contextlib import ExitStack

import numpy as np

import concourse.bass as bass
import concourse.bacc as bacc
import concourse.mybir as mybir
import concourse.tile as tile
from concourse import dve_ops
from concourse.bass_utils import run_bass_kernel_spmd
from concourse.dve_spec import Spec, Src0, Src1, _has_src1, lower, sq
from concourse.dve_uop import DveOpSpec

N_CORES = 8
N_POINTS = 16777216          # total points
PTS_PER_CORE = N_POINTS // N_CORES          # 2_097_152
ELEMS_PER_CORE = PTS_PER_CORE * 2           # 4_194_304 floats per tensor
P = 128                                      # SBUF partitions
M = ELEMS_PER_CORE // P                      # 32768 floats per partition
F = 2048                                     # tile free size per tensor
NTILES = M // F                              # 16

_cache = {}


def _register_sqdiff():
    """Register a custom DVE op out = (in0 - in1)^2 so the subtract+square
    is one Vector instruction (no ScalarE hop inside the per-tile chain)."""
    name = "SQDIFF_DIST_ANT"
    for op in dve_ops.OPS:
        if op.name == name:
            return op
    spec = Spec(
        body=sq(Src0 - Src1),
        reference=lambda in0, in1, s0, s1, imm2: (
            (in0.astype(np.float32) - in1) ** 2
        ).astype(np.float32),
    )
    row = max(dve_ops._SUB_OPCODE_FOR_NAME.values()) + 1
    assert row < 0x20
    shas = {}
    for ver in ("v3", "v4"):
        uops = lower(spec, ver=ver)
        shas[ver] = DveOpSpec(
            name=name, opcode=row, uops=uops, rd1_en=_has_src1(spec)
        ).sha(ver)
    op = dve_ops.DveOp(name, spec, subdim=False, uops_sha=shas)
    dve_ops.OPS.append(op)
    dve_ops._SUB_OPCODE_FOR_NAME[name] = row
    dve_ops.CUSTOM_DVE_SPECS[name] = spec
    return op


_SQDIFF = _register_sqdiff()


def _build(m=M, f=F):
    """Build the per-core Bass program. m = floats per partition per tensor."""
    ntiles = m // f
    fp32 = mybir.dt.float32
    nc = bacc.Bacc(
        "TRN2", target_bir_lowering=False, debug=False, num_devices=N_CORES
    )
    x_in = nc.declare_dram_parameter("x", [P, 2 * m], fp32, isOutput=False)
    out = nc.declare_dram_parameter("o", [P, 1], fp32, isOutput=True)

    with tile.TileContext(nc) as tc:
        with (
            tc.tile_pool(name="inp", bufs=5) as inp,
            tc.tile_pool(name="work", bufs=4) as work,
            tc.tile_pool(name="res", bufs=1) as res,
        ):
            acc = res.tile([P, ntiles], fp32, tag="acc")
            for i in range(ntiles):
                xt = inp.tile([P, 2 * f], fp32, tag="x")
                nc.sync.dma_start(out=xt[:], in_=x_in[:, bass.ts(i, 2 * f)])

                sq = work.tile([P, f], fp32, tag="sq")
                nc.vector._custom_dve(
                    _SQDIFF, out=sq[:], in0=xt[:, :f], in1=xt[:, f:]
                )
                ps = work.tile([P, f // 2], fp32, tag="ps")
                nc.vector.tensor_add(ps[:], sq[:, 0::2], sq[:, 1::2])
                nc.scalar.activation(
                    ps[:], ps[:], mybir.ActivationFunctionType.Sqrt,
                    accum_out=acc[:, i : i + 1],
                )
            total = res.tile([P, 1], fp32, tag="total")
            nc.vector.reduce_sum(total[:], acc[:], axis=mybir.AxisListType.X)
            nc.sync.dma_start(out=out[:], in_=total[:])
    nc.compile()
    return nc


def _tiles(m, f, taper):
    """Tile list as (elem_offset, free_size) per tensor. With taper, the
    last full tile is split geometrically (1/2, 1/4, 1/4) so the
    end-of-stream compute chain (sqdiff+pairadd+sqrt on the final tile)
    shrinks with it."""
    ntiles = m // f
    out = [(i * f, f) for i in range(ntiles)]
    if taper and ntiles >= 2 and f % 4 == 0:
        off, sz = out.pop()
        h, q = sz // 2, sz // 4
        out += [(off, h), (off + h, q), (off + h + q, q)]
    return out


def _build_raw(m=M, f=F, nb=5, pb=2, out_wait=True, lean=False, taper=False):
    """Raw bacc build (no TileContext): hand-rolled semaphores, cheap tail.

    Engines: Sync issues input DMAs (HWDGE ring, FIFO completion order),
    Vector runs sqdiff + pair-add, Scalar runs sqrt with accumulate.
    The output is the acc column vector per tile; the host does the final
    cross-tile/cross-partition sum (no on-chip reduce on the tail path).
    """
    tiles = _tiles(m, f, taper)
    T = len(tiles)
    if lean:
        # dedicated ps slot per tile (no WAR waits) and rely on the DVE
        # pipeline's output-hazard drain for same-engine RAW (no self waits)
        pb = T
    fp32 = mybir.dt.float32
    nc = bacc.Bacc(
        "TRN2", target_bir_lowering=False, debug=False, num_devices=N_CORES,
        enable_partition_id=False,
    )
    x_in = nc.declare_dram_parameter("x", [P, 2 * m], fp32, isOutput=False)
    out = nc.declare_dram_parameter("o", [P, T], fp32, isOutput=True)
    with ExitStack() as ctx:
        xt = [
            ctx.enter_context(nc.sbuf_tensor(f"xt{j}", [P, 2 * f], fp32))
            for j in range(nb)
        ]
        sqt = [
            ctx.enter_context(nc.sbuf_tensor(f"sq{j}", [P, f], fp32))
            for j in range(2)
        ]
        ps = [
            ctx.enter_context(nc.sbuf_tensor(f"ps{j}", [P, f // 2], fp32))
            for j in range(pb)
        ]
        acc = ctx.enter_context(nc.sbuf_tensor("acc", [P, T], fp32))
        dma_sems = [
            ctx.enter_context(nc.semaphore(f"dma_sem{j}")) for j in range(nb)
        ]
        out_sem = ctx.enter_context(nc.semaphore("out_sem"))
        vec_sem = ctx.enter_context(nc.semaphore("vec_sem"))
        act_sem = ctx.enter_context(nc.semaphore("act_sem"))

        with nc.Block(no_gpsimd_drain=True) as block:

            @block.sync
            def _(sync):
                for i, (off, sz) in enumerate(tiles):
                    if i >= nb:
                        # xt slot free once sqdiff of tile i-nb has read it
                        sync.wait_ge(vec_sem, 2 * (i - nb) + 1)
                    sync.dma_start(
                        xt[i % nb][:, : 2 * sz],
                        x_in[:, 2 * off : 2 * (off + sz)],
                    ).then_inc(dma_sems[i % nb], 16)
                sync.wait_ge(act_sem, T)
                sync.dma_start(out[:], acc[:]).then_inc(out_sem, 16)
                if out_wait:
                    sync.wait_ge(out_sem, 16)

            @block.vector
            def _(vector):
                for i, (off, sz) in enumerate(tiles):
                    vector.wait_ge(dma_sems[i % nb], 16 * (i // nb + 1))
                    nc.vector._custom_dve(
                        _SQDIFF,
                        out=sqt[i % 2][:, :sz],
                        in0=xt[i % nb][:, :sz],
                        in1=xt[i % nb][:, sz : 2 * sz],
                    ).then_inc(vec_sem, 1)
                    if not lean:
                        # same-engine RAW on sq (DVE pipe); HW drains this
                        # anyway, but CoreSim's race detector wants the sem
                        vector.wait_ge(vec_sem, 2 * i + 1)
                    if i >= pb:
                        # ps slot free once sqrt of tile i-pb has consumed it
                        vector.wait_ge(act_sem, i - pb + 1)
                    nc.vector.tensor_add(
                        ps[i % pb][:, : sz // 2],
                        sqt[i % 2][:, 0 : sz : 2],
                        sqt[i % 2][:, 1 : sz : 2],
                    ).then_inc(vec_sem, 1)

            @block.scalar
            def _(scalar):
                for i, (off, sz) in enumerate(tiles):
                    scalar.wait_ge(vec_sem, 2 * (i + 1))
                    nc.scalar.activation(
                        ps[i % pb][:, : sz // 2],
                        ps[i % pb][:, : sz // 2],
                        mybir.ActivationFunctionType.Sqrt,
                        accum_out=acc[:, i : i + 1],
                    ).then_inc(act_sem, 1)

    nc.compile()
    return nc


def _pack(preds, targets, m, f, n_cores, taper=False):
    """[N,2]x2 f32 -> per-core interleaved [n_cores, P, 2m]: for each tile
    (off, sz), the p-chunk then the t-chunk, matching the kernel's slicing."""
    p3 = np.ascontiguousarray(preds, dtype=np.float32).reshape(n_cores, P, m)
    t3 = np.ascontiguousarray(targets, dtype=np.float32).reshape(n_cores, P, m)
    x = np.empty((n_cores, P, 2 * m), dtype=np.float32)
    for off, sz in _tiles(m, f, taper):
        x[:, :, 2 * off : 2 * off + sz] = p3[:, :, off : off + sz]
        x[:, :, 2 * off + sz : 2 * (off + sz)] = t3[:, :, off : off + sz]
    return x


def _run(preds, targets, m=M, f=F, n_cores=N_CORES, raw=True, nb=5, pb=2,
         out_wait=False, lean=False, taper=True, **run_kwargs):
    """Shard, run on hardware, return (partials [n_cores,128,ncols], results)."""
    key = (m, f, raw, nb, pb, out_wait, lean, taper)
    if key not in _cache:
        _cache[key] = (
            _build_raw(m, f, nb=nb, pb=pb, out_wait=out_wait, lean=lean,
                       taper=taper)
            if raw
            else _build(m, f)
        )
    nc = _cache[key]
    x = _pack(preds, targets, m, f, n_cores, taper=taper and raw)
    in_maps = [{"x": x[c]} for c in range(n_cores)]
    r = run_bass_kernel_spmd(nc, in_maps, core_ids=list(range(n_cores)), **run_kwargs)
    partials = np.stack([r.results[c]["o"] for c in range(n_cores)])
    return partials, r


def kernel(preds, targets):
    import os

    # Force tracing off: the NTFF profile hook isn't importable in a bare
    # container and BASS_TRACE=1 in the environment would crash the run.
    prev = os.environ.get("BASS_NEVER_TRACE")
    os.environ["BASS_NEVER_TRACE"] = "1"
    try:
        partials, _ = _run(preds, targets)
    finally:
        if prev is None:
            os.environ.pop("BASS_NEVER_TRACE", None)
        else:
            os.environ["BASS_NEVER_TRACE"] = prev
    n = preds.shape[0]
    loss = partials.astype(np.float64).sum() / np.float64(n + 1)
    return np.float32(loss)

